# revision 4
# baseline (speedup 1.0000x reference)
"""GRU decoder kernel for 8 trn2 NeuronCores.

Algorithm notes (derivation from the reference GruDecoder):
  x_{t+1} = y_t = h_{t+1} @ W_fc.T + b_fc, so the input-path matmul folds into
  the recurrence:  gi_t = h_t @ (W_ih @ W_fc).T + (b_ih + W_ih @ b_fc)  (t>=1).
  r/z gates use gi+gh, so those rows of the folded matrix and W_hh are summed
  host-side; the n-gate keeps gi_n / gh_n separate (r multiplies only gh_n).
  Per step this leaves ONE [B,1024] @ [1024, 4*1024] matmul + elementwise.

Sharding: model-parallel over the hidden dim. Core k owns hidden slice
  J_k = [128k, 128k+128): it computes r/z/n/h_new for those 128 hidden dims
  for the FULL batch of 256 (so the PE streams N=256 per weight tile), then an
  AllGather rebuilds the full h_{t+1}^T [1024, 256] on every core. The output
  projection y_t = h_{t+1} @ W_fc.T + b_fc is computed from the gathered h
  with core k owning output columns [96k, 96k+96).
"""

import os
import sys

sys.path.insert(0, "/opt/trn_rl_repo")

import numpy as np

H = 1024
OUT = 768
B = 256
T = int(os.environ.get("GRU_T", "256"))
NCORES = 8
MSLICE = 4 * 128  # per-core folded gate rows (r,z,ni,nh) x 128 hidden dims
OSLICE = OUT // NCORES  # 96 output cols per core
K_REC = H // 128  # 8 K-tiles for the recurrence matmul
K_0 = (OUT + H) // 128  # 14 K-tiles for the step-0 matmul ([x0; h0])

_cache = {}


def _build_program():
    import concourse.mybir as mybir
    from concourse import bacc, tile

    dt = mybir.dt
    AF = mybir.ActivationFunctionType
    RG = [list(range(NCORES))]

    nc = bacc.Bacc(num_devices=NCORES)

    w_rec_d = nc.dram_tensor("w_rec", [128, K_REC, MSLICE], dt.bfloat16, kind="ExternalInput")
    w0_d = nc.dram_tensor("w0", [128, K_0, MSLICE], dt.bfloat16, kind="ExternalInput")
    wfc_d = nc.dram_tensor("wfc", [128, K_REC, OSLICE], dt.bfloat16, kind="ExternalInput")
    rhs0_d = nc.dram_tensor("rhs0", [128, K_0, B], dt.bfloat16, kind="ExternalInput")
    h0own_d = nc.dram_tensor("h0own", [128, B], dt.float32, kind="ExternalInput")
    biasS_d = nc.dram_tensor("biasS", [128, 4], dt.float32, kind="ExternalInput")
    bias0_d = nc.dram_tensor("bias0", [128, 4], dt.float32, kind="ExternalInput")
    bfc_d = nc.dram_tensor("bfc", [OSLICE, 1], dt.float32, kind="ExternalInput")
    out_d = nc.dram_tensor("out", [T, OSLICE, B], dt.float32, kind="ExternalOutput")

    with tile.TileContext(nc) as tc:
        with (
            tc.tile_pool(name="wp", bufs=1) as wp,
            tc.tile_pool(name="hp", bufs=3) as hp,
            tc.tile_pool(name="ep", bufs=2) as ep,
            tc.tile_pool(name="pp", bufs=1, space="PSUM") as pp,
            tc.tile_pool(name="yp", bufs=2, space="PSUM") as yp,
            tc.tile_pool(name="dp", bufs=2, space="DRAM") as dp,
        ):
            wrec_sb = wp.tile([128, K_REC, MSLICE], dt.bfloat16)
            nc.sync.dma_start(wrec_sb[:], w_rec_d[:])
            w0_sb = wp.tile([128, K_0, MSLICE], dt.bfloat16)
            nc.sync.dma_start(w0_sb[:], w0_d[:])
            wfc_sb = wp.tile([128, K_REC, OSLICE], dt.bfloat16)
            nc.sync.dma_start(wfc_sb[:], wfc_d[:])
            rhs0_sb = wp.tile([128, K_0, B], dt.bfloat16)
            nc.sync.dma_start(rhs0_sb[:], rhs0_d[:])
            biasS_sb = wp.tile([128, 4], dt.float32)
            nc.sync.dma_start(biasS_sb[:], biasS_d[:])
            bias0_sb = wp.tile([128, 4], dt.float32)
            nc.sync.dma_start(bias0_sb[:], bias0_d[:])
            bfc_sb = wp.tile([OSLICE, 1], dt.float32)
            nc.sync.dma_start(bfc_sb[:], bfc_d[:])

            h_f32 = hp.tile([128, B], dt.float32, tag="hf32")
            nc.sync.dma_start(h_f32[:], h0own_d[:])

            h_all = None
            for t in range(T):
                if t == 0:
                    nk, lhs, rhs, bias = K_0, w0_sb, rhs0_sb, bias0_sb
                else:
                    nk, lhs, rhs, bias = K_REC, wrec_sb, h_all, biasS_sb

                # gate m-blocks in W layout: 0=r, 1=z, 2=n_i, 3=n_h.
                # Compute order r, z, n_h, n_i so the elementwise chain can
                # start on r/z while the PE finishes the n blocks.
                Pr = pp.tile([128, B], dt.float32, tag="pr")
                Pz = pp.tile([128, B], dt.float32, tag="pz")
                Pni = pp.tile([128, B], dt.float32, tag="pni")
                Pnh = pp.tile([128, B], dt.float32, tag="pnh")
                for P, m in ((Pr, 0), (Pz, 1), (Pnh, 3), (Pni, 2)):
                    for kt in range(nk):
                        nc.tensor.matmul(
                            P[:],
                            lhs[:, kt, m * 128 : (m + 1) * 128],
                            rhs[:, kt, :],
                            start=(kt == 0),
                            stop=(kt == nk - 1),
                        )

                r = ep.tile([128, B], dt.float32, tag="r")
                nc.scalar.activation(r[:], Pr[:], AF.Sigmoid, bias=bias[:, 0:1])
                z = ep.tile([128, B], dt.float32, tag="z")
                nc.scalar.activation(z[:], Pz[:], AF.Sigmoid, bias=bias[:, 1:2])
                t1 = ep.tile([128, B], dt.float32, tag="t1")
                nc.vector.tensor_scalar_add(t1[:], Pnh[:], bias[:, 3:4])
                t2 = ep.tile([128, B], dt.float32, tag="t2")
                nc.vector.tensor_mul(t2[:], r[:], t1[:])
                t3 = ep.tile([128, B], dt.float32, tag="t3")
                nc.vector.tensor_add(t3[:], t2[:], Pni[:])
                n = ep.tile([128, B], dt.float32, tag="n")
                nc.scalar.activation(n[:], t3[:], AF.Tanh, bias=bias[:, 2:3])
                d = ep.tile([128, B], dt.float32, tag="d")
                nc.vector.tensor_sub(d[:], h_f32[:], n[:])
                zd = ep.tile([128, B], dt.float32, tag="zd")
                nc.vector.tensor_mul(zd[:], z[:], d[:])
                h_new = hp.tile([128, B], dt.float32, tag="hf32")
                nc.vector.tensor_add(h_new[:], n[:], zd[:])
                h_send = ep.tile([128, B], dt.bfloat16, tag="hs")
                nc.vector.tensor_copy(h_send[:], h_new[:])

                cc_in = dp.tile([128, B], dt.bfloat16, tag="cin")
                nc.sync.dma_start(cc_in[:], h_send[:])
                cc_out = dp.tile([NCORES * 128, B], dt.bfloat16, tag="cout")
                if os.environ.get("SKIP_CC", "0") == "1":
                    # timing diagnostic only: wrong results, no collective
                    nc.sync.dma_start(cc_out[0:128, :], cc_in[:])
                else:
                    nc.gpsimd.collective_compute(
                        "AllGather",
                        mybir.AluOpType.bypass,
                        replica_groups=RG,
                        ins=[cc_in.opt()],
                        outs=[cc_out.opt()],
                    )
                h_all = hp.tile([128, K_REC, B], dt.bfloat16, tag="hall")
                nc.sync.dma_start(
                    h_all[:], cc_out[:].rearrange("(k p) n -> p k n", p=128)
                )
                h_f32 = h_new

                # Output projection for step t: y_t = h_{t+1} @ W_fc.T + b_fc
                Py = yp.tile([OSLICE, B], dt.float32, tag="py")
                for kt in range(K_REC):
                    nc.tensor.matmul(
                        Py[:],
                        wfc_sb[:, kt, :],
                        h_all[:, kt, :],
                        start=(kt == 0),
                        stop=(kt == K_REC - 1),
                    )
                y_sb = ep.tile([OSLICE, B], dt.float32, tag="ysb")
                nc.scalar.activation(y_sb[:], Py[:], AF.Identity, bias=bfc_sb[:])
                nc.sync.dma_start(out_d[t], y_sb[:])

    nc.compile()
    return nc


def _prep_inputs(src, hidden, W_ih, W_hh, b_ih, b_hh, W_fc, b_fc):
    from ml_dtypes import bfloat16

    f32 = np.float32
    src = np.asarray(src, f32)
    hidden = np.asarray(hidden, f32)
    W_ih = np.asarray(W_ih, f32)
    W_hh = np.asarray(W_hh, f32)
    b_ih = np.asarray(b_ih, f32)
    b_hh = np.asarray(b_hh, f32)
    W_fc = np.asarray(W_fc, f32)
    b_fc = np.asarray(b_fc, f32)

    x0 = src[0]  # [B, OUT]
    h0 = hidden[0]  # [B, H]

    W_comb = W_ih @ W_fc  # [3H, H]
    b_comb = b_ih + W_ih @ b_fc  # [3H]

    def to_ktiles(lhsT, m):  # [K, m] -> [128, K/128, m]
        k = lhsT.shape[0] // 128
        return np.ascontiguousarray(
            lhsT.reshape(k, 128, m).transpose(1, 0, 2)
        ).astype(bfloat16)

    in_maps = []
    for c in range(NCORES):
        Jk = slice(128 * c, 128 * c + 128)
        Zk = slice(H + 128 * c, H + 128 * c + 128)
        Nk = slice(2 * H + 128 * c, 2 * H + 128 * c + 128)
        Ok = slice(OSLICE * c, OSLICE * c + OSLICE)

        W_rec = np.concatenate(
            [
                W_comb[Jk] + W_hh[Jk],
                W_comb[Zk] + W_hh[Zk],
                W_comb[Nk],
                W_hh[Nk],
            ],
            axis=0,
        )  # [512, H]

        W0 = np.zeros((MSLICE, OUT + H), f32)
        W0[0:128, :OUT] = W_ih[Jk]
        W0[0:128, OUT:] = W_hh[Jk]
        W0[128:256, :OUT] = W_ih[Zk]
        W0[128:256, OUT:] = W_hh[Zk]
        W0[256:384, :OUT] = W_ih[Nk]
        W0[384:512, OUT:] = W_hh[Nk]

        rhs0 = np.concatenate([x0, h0], axis=1).T  # [OUT+H, B]

        biasS = np.stack(
            [
                b_comb[Jk] + b_hh[Jk],
                b_comb[Zk] + b_hh[Zk],
                b_comb[Nk],
                b_hh[Nk],
            ],
            axis=1,
        )  # [128, 4]
        bias0 = np.stack(
            [
                b_ih[Jk] + b_hh[Jk],
                b_ih[Zk] + b_hh[Zk],
                b_ih[Nk],
                b_hh[Nk],
            ],
            axis=1,
        )

        in_maps.append(
            {
                "w_rec": to_ktiles(W_rec.T, MSLICE),
                "w0": to_ktiles(W0.T, MSLICE),
                "wfc": to_ktiles(np.ascontiguousarray(W_fc[Ok]).T, OSLICE),
                "rhs0": to_ktiles(rhs0, B),
                "h0own": np.ascontiguousarray(h0[:, Jk].T),
                "biasS": np.ascontiguousarray(biasS),
                "bias0": np.ascontiguousarray(bias0),
                "bfc": np.ascontiguousarray(b_fc[Ok].reshape(OSLICE, 1)),
            }
        )
    return in_maps


def kernel(src, tgt, hidden, W_ih, W_hh, b_ih, b_hh, W_fc, b_fc, **_unused):
    from concourse import bass_utils

    if "nc" not in _cache:
        _cache["nc"] = _build_program()
    nc = _cache["nc"]

    in_maps = _prep_inputs(src, hidden, W_ih, W_hh, b_ih, b_hh, W_fc, b_fc)
    res = bass_utils.run_bass_kernel_spmd(
        nc, in_maps, core_ids=list(range(NCORES))
    )
    # per-core out: [T, 96, B] -> full [T, B, OUT]
    outs = [np.asarray(r["out"]) for r in res.results]
    full = np.concatenate([o.transpose(0, 2, 1) for o in outs], axis=2)
    return np.ascontiguousarray(full.astype(np.float32))
